# revision 1
# baseline (speedup 1.0000x reference)
"""MoE FFN (SwiGLU, top-2 of 8 experts) on 8 Trainium2 NeuronCores.

Strategy (expert-parallel, per sharding hint):
  - Router is replicated bit-exactly with the reference via eager jnp ops on
    the default jax backend; the top-2 selection and combine weights therefore
    match the reference's routing decisions exactly.
  - Token dispatch (the "all-to-all") happens host-side as the sharding step:
    each of the 8 cores receives exactly the tokens routed to its expert
    (padded to a common capacity C), with the expert's weights pre-transposed
    into PE-friendly tiled layouts.
  - Each core runs a dense SwiGLU FFN over its C tokens in float32r (fp32
    storage, full-rate PE) with fp32 PSUM accumulation.
  - The up-projection input is pre-scaled by the per-token combine weight, so
    the core's output is already the weighted expert contribution; the host
    scatter-adds the 8 outputs back into token order (expert order preserved).

Computation per core e (feature-on-partition layout, C gathered tokens):
    g.T = WgT.T @ xTg          [I, C]
    u.T = WuT.T @ (w * x).T    [I, C]
    a.T = silu(g.T) * u.T      [I, C]
    d.T = WdT.T @ a.T          [H, C]   -> outT (scatter-added on host)
"""

from contextlib import ExitStack

import numpy as np

B, S, H, I, E, TOPK = 2, 1024, 1024, 2048, 8, 2
N = B * S
KP = 128
NH = H // KP   # 8 h-tiles
NI = I // KP   # 16 i-tiles
N_CORES = 8

_COMPILED = {}  # C -> bass module


# ---------------------------------------------------------------------------
# Bass/Tile kernel
# ---------------------------------------------------------------------------

def _chunk_sizes(C, max_chunk=512):
    out, n0 = [], 0
    while n0 < C:
        sz = min(max_chunk, C - n0)
        out.append((n0, sz))
        n0 += sz
    return out


# This walrus build accepts only one semaphore wait on most engine
# instructions; Tile can emit more. Move the excess onto preceding sequencer
# NoOps on the same engine (equivalent blocking, engines execute in order).
_WAIT_LIMITS = {
    "InstTensorTensor": 1,
    "InstTensorCopy": 1,
    "InstActivation": 1,
    "InstMatmult": 1,
    "InstLdweights": 1,
    "InstMemset": 1,
    "InstTensorReduce": 1,
    "InstTensorScalarPtr": 1,
    "InstIota": 1,
    "InstDMACopy": 1,
    "InstDrain": 1,
    "InstEventSemaphore": 1,
}


def _legalize_waits(nc, mybir):
    nop_id = 0
    for f in nc.m.functions:
        for b in f.blocks:
            insts = list(b.instructions)
            out, changed = [], False
            for i in insts:
                lim = _WAIT_LIMITS.get(type(i).__name__)
                si = i.sync_info
                if lim is not None and si is not None and si.on_wait and len(si.on_wait) > lim:
                    waits = list(si.on_wait)
                    for w in waits[:-lim]:
                        nop_id += 1
                        nop = mybir.InstNoOp(name=f"I-waitnop-{nop_id}")
                        nop.engine = i.engine
                        nop.sync_info = mybir.SyncInfo(on_wait=[w], on_update=[])
                        out.append(nop)
                    si.on_wait = waits[-lim:]
                    changed = True
                out.append(i)
            if changed:
                b.instructions = out


def _build_moe_ffn(C):
    import concourse.bass as bass
    import concourse.mybir as mybir
    import concourse.tile as tile

    F32 = mybir.dt.float32
    F32R = mybir.dt.float32r
    AF = mybir.ActivationFunctionType

    nc = bass.Bass()
    xg = nc.declare_dram_parameter("xg", [NH, KP, C], F32R, isOutput=False)
    xu = nc.declare_dram_parameter("xu", [NH, KP, C], F32R, isOutput=False)
    wg = nc.declare_dram_parameter("wg", [NI, KP, H], F32R, isOutput=False)
    wu = nc.declare_dram_parameter("wu", [NI, KP, H], F32R, isOutput=False)
    wd = nc.declare_dram_parameter("wd", [NH, KP, I], F32R, isOutput=False)
    outT = nc.declare_dram_parameter("outT", [NH, KP, C], F32, isOutput=True)

    chunks = _chunk_sizes(C)

    with tile.TileContext(nc) as tc, ExitStack() as ctx:
        xpool = ctx.enter_context(tc.tile_pool(name="x", bufs=1))
        apool = ctx.enter_context(tc.tile_pool(name="act", bufs=1))
        wgp = ctx.enter_context(tc.tile_pool(name="wg", bufs=2))
        wup = ctx.enter_context(tc.tile_pool(name="wu", bufs=2))
        wdp = ctx.enter_context(tc.tile_pool(name="wd", bufs=2))
        tmp = ctx.enter_context(tc.tile_pool(name="tmp", bufs=3))
        opool = ctx.enter_context(tc.tile_pool(name="o", bufs=3))
        pgp = ctx.enter_context(tc.tile_pool(name="pg", bufs=2, space="PSUM"))
        pup = ctx.enter_context(tc.tile_pool(name="pu", bufs=2, space="PSUM"))
        pdp = ctx.enter_context(tc.tile_pool(name="pd", bufs=3, space="PSUM"))

        xg_sb = xpool.tile([KP, NH * C], F32R, tag="xg")
        xu_sb = xpool.tile([KP, NH * C], F32R, tag="xu")
        for ko in range(NH):
            nc.sync.dma_start(out=xg_sb[:, ko * C:(ko + 1) * C], in_=xg[ko])
            nc.sync.dma_start(out=xu_sb[:, ko * C:(ko + 1) * C], in_=xu[ko])

        act_sb = apool.tile([KP, NI * C], F32R, tag="a")

        # Phase A: gate/up projections + SwiGLU, one i-tile at a time
        for t in range(NI):
            wg_sb = wgp.tile([KP, H], F32R, tag="wg")
            nc.sync.dma_start(out=wg_sb, in_=wg[t])
            wu_sb = wup.tile([KP, H], F32R, tag="wu")
            nc.sync.dma_start(out=wu_sb, in_=wu[t])
            for (n0, nsz) in chunks:
                pg = pgp.tile([KP, nsz], mybir.dt.float32, tag="pg")
                pu = pup.tile([KP, nsz], mybir.dt.float32, tag="pu")
                for ko in range(NH):
                    nc.tensor.matmul(
                        pg,
                        lhsT=wg_sb[:, ko * KP:(ko + 1) * KP],
                        rhs=xg_sb[:, ko * C + n0:ko * C + n0 + nsz],
                        start=(ko == 0), stop=(ko == NH - 1),
                    )
                for ko in range(NH):
                    nc.tensor.matmul(
                        pu,
                        lhsT=wu_sb[:, ko * KP:(ko + 1) * KP],
                        rhs=xu_sb[:, ko * C + n0:ko * C + n0 + nsz],
                        start=(ko == 0), stop=(ko == NH - 1),
                    )
                # Both mul operands produced by ACT so the DVE mul needs only
                # one semaphore wait (walrus limit).
                silu_t = tmp.tile([KP, nsz], mybir.dt.float32, tag="silu")
                nc.scalar.activation(silu_t, pg, AF.Silu)
                u_t = tmp.tile([KP, nsz], mybir.dt.float32, tag="ut")
                nc.scalar.activation(u_t, pu, AF.Copy)
                nc.vector.tensor_mul(act_sb[:, t * C + n0:t * C + n0 + nsz], silu_t, u_t)

        # Phase B: down projection
        for ho in range(NH):
            wd_sb = wdp.tile([KP, I], F32R, tag="wd")
            nc.sync.dma_start(out=wd_sb, in_=wd[ho])
            for (n0, nsz) in chunks:
                pd = pdp.tile([KP, nsz], mybir.dt.float32, tag="pd")
                for ko in range(NI):
                    nc.tensor.matmul(
                        pd,
                        lhsT=wd_sb[:, ko * KP:(ko + 1) * KP],
                        rhs=act_sb[:, ko * C + n0:ko * C + n0 + nsz],
                        start=(ko == 0), stop=(ko == NI - 1),
                    )
                o_sb = opool.tile([KP, nsz], mybir.dt.float32, tag="o")
                nc.vector.tensor_copy(o_sb, pd)
                nc.sync.dma_start(out=outT[ho, :, n0:n0 + nsz], in_=o_sb)

    _legalize_waits(nc, mybir)
    return nc


def _get_compiled(C):
    if C not in _COMPILED:
        _COMPILED[C] = _build_moe_ffn(C)
    return _COMPILED[C]


# ---------------------------------------------------------------------------
# Host-side layouts
# ---------------------------------------------------------------------------

def _proj_layout(W, nt):
    """W: [rows=nt*128, cols=nk*128] -> [nt, 128, cols]; slab t column-block ko
    holds W[t-block, ko-block].T so each [128,128] block is a matmul lhsT."""
    rows, cols = W.shape
    nk = cols // KP
    W4 = W.reshape(nt, KP, nk, KP)           # [t, i, ko, p]
    return np.ascontiguousarray(W4.transpose(0, 3, 2, 1).reshape(nt, KP, cols))


def _x_layout(x_sel):
    """x_sel: [C, H] -> [NH, 128, C] (x_sel.T tiled by h)."""
    C = x_sel.shape[0]
    return np.ascontiguousarray(x_sel.T.reshape(NH, KP, C))


# ---------------------------------------------------------------------------
# Main entry
# ---------------------------------------------------------------------------

def run_moe(x, W_router, W_gate, W_up, W_down, profile=False):
    import jax
    import jax.numpy as jnp
    from concourse.bass_utils import run_bass_kernel_spmd

    x = np.asarray(x, dtype=np.float32)
    W_router = np.asarray(W_router, dtype=np.float32)
    W_gate = np.asarray(W_gate, dtype=np.float32)
    W_up = np.asarray(W_up, dtype=np.float32)
    W_down = np.asarray(W_down, dtype=np.float32)

    b, s, h = x.shape
    n = b * s
    x_flat = x.reshape(n, h)

    # --- Router: replicate the reference's ops eagerly on the default backend
    # so the discrete top-k selection matches it bit-for-bit.
    xj = jnp.asarray(x_flat)
    probs = jax.nn.softmax(xj @ jnp.asarray(W_router).T, axis=-1)
    topk_w, topk_idx = jax.lax.top_k(probs, TOPK)
    topk_w = topk_w / topk_w.sum(axis=-1, keepdims=True)
    onehot = jax.nn.one_hot(topk_idx, E, dtype=xj.dtype)
    counts = onehot.sum(axis=1)
    aux_loss = jnp.var(counts.mean(axis=0), ddof=1) * E

    topk_idx_np = np.asarray(topk_idx)       # [N, 2] int32
    topk_w_np = np.asarray(topk_w)           # [N, 2] f32 (renormalized)

    # --- Dispatch (host-side sharding): per-expert token lists + capacity pad
    idx_lists, w_lists = [], []
    for e in range(E):
        sel = np.nonzero((topk_idx_np == e).any(axis=1))[0]
        we = np.where(topk_idx_np[sel, 0] == e,
                      topk_w_np[sel, 0], topk_w_np[sel, 1]).astype(np.float32)
        idx_lists.append(sel)
        w_lists.append(we)
    max_cnt = max(len(ix) for ix in idx_lists)
    C = max(KP, ((max_cnt + KP - 1) // KP) * KP)

    nc = _get_compiled(C)

    in_maps = []
    for e in range(E):
        sel, we = idx_lists[e], w_lists[e]
        cnt = len(sel)
        x_sel = np.zeros((C, H), dtype=np.float32)
        x_sel[:cnt] = x_flat[sel]
        wcol = np.zeros((C, 1), dtype=np.float32)
        wcol[:cnt, 0] = we
        Wg_e, Wu_e, Wd_e = W_gate[e], W_up[e], W_down[e]
        in_maps.append({
            "xg": _x_layout(x_sel),
            "xu": _x_layout(x_sel * wcol),
            "wg": _proj_layout(Wg_e, NI),
            "wu": _proj_layout(Wu_e, NI),
            "wd": _proj_layout(Wd_e, NH),
        })

    prof_times = None
    if profile:
        import profiling
        results, prof_times, neff_dir = profiling.run_profiled(
            nc, in_maps, list(range(N_CORES)))
    else:
        results = run_bass_kernel_spmd(nc, in_maps, list(range(N_CORES))).results

    # --- Combine (host-side unshard): scatter-add in expert order
    out_flat = np.zeros((n, H), dtype=np.float32)
    for e in range(E):
        sel = idx_lists[e]
        cnt = len(sel)
        d_e = results[e]["outT"].reshape(H, C).T   # [C, H]
        out_flat[sel] += d_e[:cnt]

    out = out_flat.reshape(b, s, h)
    return (out, np.asarray(aux_loss)), prof_times


def kernel(x, W_router, W_gate, W_up, W_down):
    (out, aux_loss), _ = run_moe(x, W_router, W_gate, W_up, W_down)
    return out, aux_loss


# revision 3
# speedup vs baseline: 1.0331x; 1.0331x over previous
"""MoE FFN (SwiGLU, top-2 of 8 experts) on 8 Trainium2 NeuronCores.

Strategy (expert-parallel, per sharding hint):
  - Router is replicated bit-exactly with the reference via eager jnp ops on
    the default jax backend; the top-2 selection and combine weights therefore
    match the reference's routing decisions exactly.
  - Token dispatch (the "all-to-all") happens host-side as the sharding step:
    each of the 8 cores receives exactly the tokens routed to its expert
    (padded to a common capacity C), with the expert's weights pre-transposed
    into PE-friendly tiled layouts.
  - Each core runs a dense SwiGLU FFN over its C tokens in float32r (fp32
    storage, full-rate PE) with fp32 PSUM accumulation.
  - The up-projection input is pre-scaled by the per-token combine weight, so
    the core's output is already the weighted expert contribution; the host
    scatter-adds the 8 outputs back into token order (expert order preserved).

Computation per core e (feature-on-partition layout, C gathered tokens):
    g.T = WgT.T @ xTg          [I, C]
    u.T = WuT.T @ (w * x).T    [I, C]
    a.T = silu(g.T) * u.T      [I, C]
    d.T = WdT.T @ a.T          [H, C]   -> outT (scatter-added on host)
"""

from contextlib import ExitStack

import numpy as np

B, S, H, I, E, TOPK = 2, 1024, 1024, 2048, 8, 2
N = B * S
KP = 128
NH = H // KP   # 8 h-tiles
NI = I // KP   # 16 i-tiles
N_CORES = 8

_COMPILED = {}  # C -> bass module


# ---------------------------------------------------------------------------
# Bass/Tile kernel
# ---------------------------------------------------------------------------

def _chunk_sizes(C, max_chunk=512):
    out, n0 = [], 0
    while n0 < C:
        sz = min(max_chunk, C - n0)
        out.append((n0, sz))
        n0 += sz
    return out


# This walrus build accepts only one semaphore wait on most engine
# instructions; Tile can emit more. Move the excess onto preceding sequencer
# NoOps on the same engine (equivalent blocking, engines execute in order).
_WAIT_LIMITS = {
    "InstTensorTensor": 1,
    "InstTensorCopy": 1,
    "InstActivation": 1,
    "InstMatmult": 1,
    "InstLdweights": 1,
    "InstMemset": 1,
    "InstTensorReduce": 1,
    "InstTensorScalarPtr": 1,
    "InstIota": 1,
    "InstDMACopy": 1,
    "InstDrain": 1,
    "InstEventSemaphore": 1,
}


def _legalize_waits(nc, mybir):
    nop_id = 0
    for f in nc.m.functions:
        for b in f.blocks:
            insts = list(b.instructions)
            out, changed = [], False
            for i in insts:
                lim = _WAIT_LIMITS.get(type(i).__name__)
                si = i.sync_info
                if lim is not None and si is not None and si.on_wait and len(si.on_wait) > lim:
                    waits = list(si.on_wait)
                    for w in waits[:-lim]:
                        nop_id += 1
                        nop = mybir.InstNoOp(name=f"I-waitnop-{nop_id}")
                        nop.engine = i.engine
                        nop.sync_info = mybir.SyncInfo(on_wait=[w], on_update=[])
                        out.append(nop)
                    si.on_wait = waits[-lim:]
                    changed = True
                out.append(i)
            if changed:
                b.instructions = out


def _build_moe_ffn(C, even_chunks=True, warmup_mms=40):
    import concourse.bass as bass
    import concourse.mybir as mybir
    import concourse.tile as tile

    F32 = mybir.dt.float32
    F32R = mybir.dt.float32r
    AF = mybir.ActivationFunctionType

    nc = bass.Bass()
    xg = nc.declare_dram_parameter("xg", [NH, KP, C], F32R, isOutput=False)
    sc = nc.declare_dram_parameter("sc", [1, C], F32, isOutput=False)
    wg = nc.declare_dram_parameter("wg", [NI, KP, H], F32R, isOutput=False)
    wu = nc.declare_dram_parameter("wu", [NI, KP, H], F32R, isOutput=False)
    wd = nc.declare_dram_parameter("wd", [NH, KP, I], F32R, isOutput=False)
    outT = nc.declare_dram_parameter("outT", [NH, KP, C], F32, isOutput=True)

    if even_chunks and C > 512:
        nck = (C + 511) // 512
        base = (C // nck) // KP * KP
        szs, left = [], C
        for i in range(nck - 1):
            szs.append(base)
            left -= base
        szs.append(left)
        chunks, n0 = [], 0
        for sz in szs:
            chunks.append((n0, sz))
            n0 += sz
    else:
        chunks = _chunk_sizes(C)

    with tile.TileContext(nc) as tc, ExitStack() as ctx:
        xpool = ctx.enter_context(tc.tile_pool(name="x", bufs=1))
        apool = ctx.enter_context(tc.tile_pool(name="act", bufs=1))
        spool = ctx.enter_context(tc.tile_pool(name="scale", bufs=1))
        wgp = ctx.enter_context(tc.tile_pool(name="wg", bufs=2))
        wup = ctx.enter_context(tc.tile_pool(name="wu", bufs=2))
        wdp = ctx.enter_context(tc.tile_pool(name="wd", bufs=2))
        tmp = ctx.enter_context(tc.tile_pool(name="tmp", bufs=3))
        opool = ctx.enter_context(tc.tile_pool(name="o", bufs=3))
        pgp = ctx.enter_context(tc.tile_pool(name="pg", bufs=2, space="PSUM"))
        pup = ctx.enter_context(tc.tile_pool(name="pu", bufs=2, space="PSUM"))
        pdp = ctx.enter_context(tc.tile_pool(name="pd", bufs=3, space="PSUM"))
        pwp = ctx.enter_context(tc.tile_pool(name="pw", bufs=1, space="PSUM"))

        # PE warm-up during the initial DMA window: fp32 matmuls on memset
        # tiles keep the HAM activity window busy so real matmuls start at
        # full clock.
        if warmup_mms:
            wa = tmp.tile([KP, KP], F32, tag="wa")
            wb = tmp.tile([KP, 64], F32, tag="wb")
            nc.vector.memset(wa, 0.0)
            nc.vector.memset(wb, 0.0)
            pw = pwp.tile([KP, 64], F32, tag="pw")
            for i in range(warmup_mms):
                nc.tensor.matmul(pw, lhsT=wa, rhs=wb,
                                 start=(i == 0), stop=(i == warmup_mms - 1))

        # Weight slab for t=0 queued before the x tiles so the first matmuls
        # can start as soon as xg[0] lands.
        wg_sb0 = wgp.tile([KP, H], F32R, tag="wg")
        nc.sync.dma_start(out=wg_sb0, in_=wg[0])
        wu_sb0 = wup.tile([KP, H], F32R, tag="wu")
        nc.sync.dma_start(out=wu_sb0, in_=wu[0])

        xg_sb = xpool.tile([KP, NH * C], F32R, tag="xg")
        for ko in range(NH):
            nc.sync.dma_start(out=xg_sb[:, ko * C:(ko + 1) * C], in_=xg[ko])

        scale_sb = spool.tile([KP, C], F32, tag="sc")
        sc_b = bass.AP(tensor=sc[:].tensor, offset=0, ap=[[0, KP], [1, C]])
        nc.gpsimd.dma_start(out=scale_sb, in_=sc_b)

        act_sb = apool.tile([KP, NI * C], F32R, tag="a")

        # Phase A: gate/up projections + SwiGLU, one i-tile at a time
        for t in range(NI):
            if t == 0:
                wg_sb, wu_sb = wg_sb0, wu_sb0
            else:
                wg_sb = wgp.tile([KP, H], F32R, tag="wg")
                nc.sync.dma_start(out=wg_sb, in_=wg[t])
                wu_sb = wup.tile([KP, H], F32R, tag="wu")
                nc.sync.dma_start(out=wu_sb, in_=wu[t])
            for (n0, nsz) in chunks:
                pg = pgp.tile([KP, nsz], F32, tag="pg")
                pu = pup.tile([KP, nsz], F32, tag="pu")
                for ko in range(NH):
                    nc.tensor.matmul(
                        pg,
                        lhsT=wg_sb[:, ko * KP:(ko + 1) * KP],
                        rhs=xg_sb[:, ko * C + n0:ko * C + n0 + nsz],
                        start=(ko == 0), stop=(ko == NH - 1),
                    )
                for ko in range(NH):
                    nc.tensor.matmul(
                        pu,
                        lhsT=wu_sb[:, ko * KP:(ko + 1) * KP],
                        rhs=xg_sb[:, ko * C + n0:ko * C + n0 + nsz],
                        start=(ko == 0), stop=(ko == NH - 1),
                    )
                # Both mul operands produced by ACT so the DVE mul needs only
                # one semaphore wait (walrus limit).
                silu_t = tmp.tile([KP, nsz], F32, tag="silu")
                nc.scalar.activation(silu_t, pg, AF.Silu)
                u_t = tmp.tile([KP, nsz], F32, tag="ut")
                nc.scalar.activation(u_t, pu, AF.Copy)
                nc.vector.tensor_mul(act_sb[:, t * C + n0:t * C + n0 + nsz], silu_t, u_t)

        # Phase B: down projection, output scaled by per-token combine weight
        for ho in range(NH):
            wd_sb = wdp.tile([KP, I], F32R, tag="wd")
            nc.sync.dma_start(out=wd_sb, in_=wd[ho])
            for (n0, nsz) in chunks:
                pd = pdp.tile([KP, nsz], F32, tag="pd")
                for ko in range(NI):
                    nc.tensor.matmul(
                        pd,
                        lhsT=wd_sb[:, ko * KP:(ko + 1) * KP],
                        rhs=act_sb[:, ko * C + n0:ko * C + n0 + nsz],
                        start=(ko == 0), stop=(ko == NI - 1),
                    )
                o_sb = opool.tile([KP, nsz], F32, tag="o")
                nc.vector.tensor_mul(o_sb, pd, scale_sb[:, n0:n0 + nsz])
                nc.sync.dma_start(out=outT[ho, :, n0:n0 + nsz], in_=o_sb)

    _legalize_waits(nc, mybir)
    return nc


def _get_compiled(C):
    if C not in _COMPILED:
        _COMPILED[C] = _build_moe_ffn(C)
    return _COMPILED[C]


# ---------------------------------------------------------------------------
# Host-side layouts
# ---------------------------------------------------------------------------

def _proj_layout(W, nt):
    """W: [rows=nt*128, cols=nk*128] -> [nt, 128, cols]; slab t column-block ko
    holds W[t-block, ko-block].T so each [128,128] block is a matmul lhsT."""
    rows, cols = W.shape
    nk = cols // KP
    W4 = W.reshape(nt, KP, nk, KP)           # [t, i, ko, p]
    return np.ascontiguousarray(W4.transpose(0, 3, 2, 1).reshape(nt, KP, cols))


def _x_layout(x_sel):
    """x_sel: [C, H] -> [NH, 128, C] (x_sel.T tiled by h)."""
    C = x_sel.shape[0]
    return np.ascontiguousarray(x_sel.T.reshape(NH, KP, C))


# ---------------------------------------------------------------------------
# Main entry
# ---------------------------------------------------------------------------

def run_moe(x, W_router, W_gate, W_up, W_down, profile=False):
    import jax
    import jax.numpy as jnp
    from concourse.bass_utils import run_bass_kernel_spmd

    x = np.asarray(x, dtype=np.float32)
    W_router = np.asarray(W_router, dtype=np.float32)
    W_gate = np.asarray(W_gate, dtype=np.float32)
    W_up = np.asarray(W_up, dtype=np.float32)
    W_down = np.asarray(W_down, dtype=np.float32)

    b, s, h = x.shape
    n = b * s
    x_flat = x.reshape(n, h)

    # --- Router: replicate the reference's ops eagerly on the default backend
    # so the discrete top-k selection matches it bit-for-bit.
    xj = jnp.asarray(x_flat)
    probs = jax.nn.softmax(xj @ jnp.asarray(W_router).T, axis=-1)
    topk_w, topk_idx = jax.lax.top_k(probs, TOPK)
    topk_w = topk_w / topk_w.sum(axis=-1, keepdims=True)
    onehot = jax.nn.one_hot(topk_idx, E, dtype=xj.dtype)
    counts = onehot.sum(axis=1)
    aux_loss = jnp.var(counts.mean(axis=0), ddof=1) * E

    topk_idx_np = np.asarray(topk_idx)       # [N, 2] int32
    topk_w_np = np.asarray(topk_w)           # [N, 2] f32 (renormalized)

    # --- Dispatch (host-side sharding): per-expert token lists + capacity pad
    idx_lists, w_lists = [], []
    for e in range(E):
        sel = np.nonzero((topk_idx_np == e).any(axis=1))[0]
        we = np.where(topk_idx_np[sel, 0] == e,
                      topk_w_np[sel, 0], topk_w_np[sel, 1]).astype(np.float32)
        idx_lists.append(sel)
        w_lists.append(we)
    max_cnt = max(len(ix) for ix in idx_lists)
    C = max(KP, ((max_cnt + KP - 1) // KP) * KP)

    nc = _get_compiled(C)

    in_maps = []
    for e in range(E):
        sel, we = idx_lists[e], w_lists[e]
        cnt = len(sel)
        x_sel = np.zeros((C, H), dtype=np.float32)
        x_sel[:cnt] = x_flat[sel]
        scrow = np.zeros((1, C), dtype=np.float32)
        scrow[0, :cnt] = we
        in_maps.append({
            "xg": _x_layout(x_sel),
            "sc": scrow,
            "wg": _proj_layout(W_gate[e], NI),
            "wu": _proj_layout(W_up[e], NI),
            "wd": _proj_layout(W_down[e], NH),
        })

    prof_times = None
    if profile:
        import profiling
        results, prof_times, neff_dir = profiling.run_profiled(
            nc, in_maps, list(range(N_CORES)))
    else:
        results = run_bass_kernel_spmd(nc, in_maps, list(range(N_CORES))).results

    # --- Combine (host-side unshard): scatter-add in expert order
    out_flat = np.zeros((n, H), dtype=np.float32)
    for e in range(E):
        sel = idx_lists[e]
        cnt = len(sel)
        d_e = results[e]["outT"].reshape(H, C).T   # [C, H]
        out_flat[sel] += d_e[:cnt]

    out = out_flat.reshape(b, s, h)
    return (out, np.asarray(aux_loss)), prof_times


def kernel(x, W_router, W_gate, W_up, W_down):
    (out, aux_loss), _ = run_moe(x, W_router, W_gate, W_up, W_down)
    return out, aux_loss


# revision 4
# speedup vs baseline: 1.1623x; 1.1250x over previous
"""MoE FFN (SwiGLU, top-2 of 8 experts) on 8 Trainium2 NeuronCores.

Strategy (expert-parallel, per sharding hint):
  - Router is replicated bit-exactly with the reference via eager jnp ops on
    the default jax backend; the top-2 selection and combine weights therefore
    match the reference's routing decisions exactly.
  - Token dispatch (the "all-to-all") happens host-side as the sharding step:
    each of the 8 cores receives exactly the tokens routed to its expert
    (padded to a common capacity C), with the expert's weights pre-transposed
    into PE-friendly tiled layouts.
  - Each core runs a dense SwiGLU FFN over its C tokens in float32r (fp32
    storage, full-rate PE) with fp32 PSUM accumulation.
  - The up-projection input is pre-scaled by the per-token combine weight, so
    the core's output is already the weighted expert contribution; the host
    scatter-adds the 8 outputs back into token order (expert order preserved).

Computation per core e (feature-on-partition layout, C gathered tokens):
    g.T = WgT.T @ xTg          [I, C]
    u.T = WuT.T @ (w * x).T    [I, C]
    a.T = silu(g.T) * u.T      [I, C]
    d.T = WdT.T @ a.T          [H, C]   -> outT (scatter-added on host)
"""

from contextlib import ExitStack

import numpy as np

B, S, H, I, E, TOPK = 2, 1024, 1024, 2048, 8, 2
N = B * S
KP = 128
NH = H // KP   # 8 h-tiles
NI = I // KP   # 16 i-tiles
N_CORES = 8

_COMPILED = {}  # C -> bass module


# ---------------------------------------------------------------------------
# Bass/Tile kernel
# ---------------------------------------------------------------------------

def _chunk_sizes(C, max_chunk=512):
    out, n0 = [], 0
    while n0 < C:
        sz = min(max_chunk, C - n0)
        out.append((n0, sz))
        n0 += sz
    return out


# This walrus build accepts only one semaphore wait on most engine
# instructions; Tile can emit more. Move the excess onto preceding sequencer
# NoOps on the same engine (equivalent blocking, engines execute in order).
_WAIT_LIMITS = {
    "InstTensorTensor": 1,
    "InstTensorCopy": 1,
    "InstActivation": 1,
    "InstMatmult": 1,
    "InstLdweights": 1,
    "InstMemset": 1,
    "InstTensorReduce": 1,
    "InstTensorScalarPtr": 1,
    "InstIota": 1,
    "InstDMACopy": 1,
    "InstDrain": 1,
    "InstEventSemaphore": 1,
}


def _legalize_waits(nc, mybir):
    nop_id = 0
    for f in nc.m.functions:
        for b in f.blocks:
            insts = list(b.instructions)
            out, changed = [], False
            for i in insts:
                lim = _WAIT_LIMITS.get(type(i).__name__)
                si = i.sync_info
                if lim is not None and si is not None and si.on_wait and len(si.on_wait) > lim:
                    waits = list(si.on_wait)
                    for w in waits[:-lim]:
                        nop_id += 1
                        nop = mybir.InstNoOp(name=f"I-waitnop-{nop_id}")
                        nop.engine = i.engine
                        nop.sync_info = mybir.SyncInfo(on_wait=[w], on_update=[])
                        out.append(nop)
                    si.on_wait = waits[-lim:]
                    changed = True
                out.append(i)
            if changed:
                b.instructions = out


def _build_moe_ffn(C, even_chunks=True, warmup_mms=40):
    import concourse.bass as bass
    import concourse.mybir as mybir
    import concourse.tile as tile

    F32 = mybir.dt.float32
    F32R = mybir.dt.float32r
    AF = mybir.ActivationFunctionType

    nc = bass.Bass()
    xg = nc.declare_dram_parameter("xg", [NH, KP, C], F32R, isOutput=False)
    sc = nc.declare_dram_parameter("sc", [1, C], F32, isOutput=False)
    wg = nc.declare_dram_parameter("wg", [NI, KP, H], F32R, isOutput=False)
    wu = nc.declare_dram_parameter("wu", [NI, KP, H], F32R, isOutput=False)
    wd = nc.declare_dram_parameter("wd", [NH, KP, I], F32R, isOutput=False)
    outT = nc.declare_dram_parameter("outT", [NH, KP, C], F32, isOutput=True)

    if even_chunks and C > 512:
        nck = (C + 511) // 512
        base = (C // nck) // KP * KP
        szs, left = [], C
        for i in range(nck - 1):
            szs.append(base)
            left -= base
        szs.append(left)
        chunks, n0 = [], 0
        for sz in szs:
            chunks.append((n0, sz))
            n0 += sz
    else:
        chunks = _chunk_sizes(C)

    with tile.TileContext(nc) as tc, ExitStack() as ctx:
        xpool = ctx.enter_context(tc.tile_pool(name="x", bufs=1))
        apool = ctx.enter_context(tc.tile_pool(name="act", bufs=1))
        spool = ctx.enter_context(tc.tile_pool(name="scale", bufs=1))
        wgp = ctx.enter_context(tc.tile_pool(name="wg", bufs=2))
        wup = ctx.enter_context(tc.tile_pool(name="wu", bufs=2))
        wdp = ctx.enter_context(tc.tile_pool(name="wd", bufs=NH))
        tmp = ctx.enter_context(tc.tile_pool(name="tmp", bufs=3))
        opool = ctx.enter_context(tc.tile_pool(name="o", bufs=3))
        pgp = ctx.enter_context(tc.tile_pool(name="pg", bufs=2, space="PSUM"))
        pup = ctx.enter_context(tc.tile_pool(name="pu", bufs=2, space="PSUM"))
        pdp = ctx.enter_context(tc.tile_pool(name="pd", bufs=3, space="PSUM"))
        pwp = ctx.enter_context(tc.tile_pool(name="pw", bufs=1, space="PSUM"))

        # PE warm-up during the initial DMA window: fp32 matmuls on memset
        # tiles keep the HAM activity window busy so real matmuls start at
        # full clock.
        if warmup_mms:
            wa = tmp.tile([KP, KP], F32, tag="wa")
            wb = tmp.tile([KP, 64], F32, tag="wb")
            nc.vector.memset(wa, 0.0)
            nc.vector.memset(wb, 0.0)
            pw = pwp.tile([KP, 64], F32, tag="pw")
            for i in range(warmup_mms):
                nc.tensor.matmul(pw, lhsT=wa, rhs=wb,
                                 start=(i == 0), stop=(i == warmup_mms - 1))

        # Weight slab for t=0 queued before the x tiles so the first matmuls
        # can start as soon as xg[0] lands.
        wg_sb0 = wgp.tile([KP, H], F32R, tag="wg")
        nc.sync.dma_start(out=wg_sb0, in_=wg[0])
        wu_sb0 = wup.tile([KP, H], F32R, tag="wu")
        nc.sync.dma_start(out=wu_sb0, in_=wu[0])

        xg_sb = xpool.tile([KP, NH * C], F32R, tag="xg")
        for ko in range(NH):
            nc.sync.dma_start(out=xg_sb[:, ko * C:(ko + 1) * C], in_=xg[ko])

        scale_sb = spool.tile([KP, C], F32, tag="sc")
        sc_b = bass.AP(tensor=sc[:].tensor, offset=0, ap=[[0, KP], [1, C]])
        nc.gpsimd.dma_start(out=scale_sb, in_=sc_b)

        act_sb = apool.tile([KP, NI * C], F32R, tag="a")

        # Phase A: gate/up projections + SwiGLU, one i-tile at a time
        for t in range(NI):
            if t == 0:
                wg_sb, wu_sb = wg_sb0, wu_sb0
            else:
                wg_sb = wgp.tile([KP, H], F32R, tag="wg")
                nc.sync.dma_start(out=wg_sb, in_=wg[t])
                wu_sb = wup.tile([KP, H], F32R, tag="wu")
                nc.sync.dma_start(out=wu_sb, in_=wu[t])
            for (n0, nsz) in chunks:
                pg = pgp.tile([KP, nsz], F32, tag="pg")
                pu = pup.tile([KP, nsz], F32, tag="pu")
                for ko in range(NH):
                    nc.tensor.matmul(
                        pg,
                        lhsT=wg_sb[:, ko * KP:(ko + 1) * KP],
                        rhs=xg_sb[:, ko * C + n0:ko * C + n0 + nsz],
                        start=(ko == 0), stop=(ko == NH - 1),
                    )
                for ko in range(NH):
                    nc.tensor.matmul(
                        pu,
                        lhsT=wu_sb[:, ko * KP:(ko + 1) * KP],
                        rhs=xg_sb[:, ko * C + n0:ko * C + n0 + nsz],
                        start=(ko == 0), stop=(ko == NH - 1),
                    )
                # Both mul operands produced by ACT so the DVE mul needs only
                # one semaphore wait (walrus limit).
                silu_t = tmp.tile([KP, nsz], F32, tag="silu")
                nc.scalar.activation(silu_t, pg, AF.Silu)
                u_t = tmp.tile([KP, nsz], F32, tag="ut")
                nc.scalar.activation(u_t, pu, AF.Copy)
                nc.vector.tensor_mul(act_sb[:, t * C + n0:t * C + n0 + nsz], silu_t, u_t)

        # Phase B: down projection, output scaled by per-token combine weight.
        # All slabs prefetched (they stream in behind phase A's weight loads).
        wd_sbs = []
        for ho in range(NH):
            wd_sb = wdp.tile([KP, I], F32R, tag="wd")
            nc.sync.dma_start(out=wd_sb, in_=wd[ho])
            wd_sbs.append(wd_sb)
        for ho in range(NH):
            wd_sb = wd_sbs[ho]
            for (n0, nsz) in chunks:
                pd = pdp.tile([KP, nsz], F32, tag="pd")
                for ko in range(NI):
                    nc.tensor.matmul(
                        pd,
                        lhsT=wd_sb[:, ko * KP:(ko + 1) * KP],
                        rhs=act_sb[:, ko * C + n0:ko * C + n0 + nsz],
                        start=(ko == 0), stop=(ko == NI - 1),
                    )
                o_sb = opool.tile([KP, nsz], F32, tag="o")
                nc.vector.tensor_mul(o_sb, pd, scale_sb[:, n0:n0 + nsz])
                nc.sync.dma_start(out=outT[ho, :, n0:n0 + nsz], in_=o_sb)

    _legalize_waits(nc, mybir)
    return nc


def _get_compiled(C):
    if C not in _COMPILED:
        _COMPILED[C] = _build_moe_ffn(C)
    return _COMPILED[C]


# ---------------------------------------------------------------------------
# Host-side layouts
# ---------------------------------------------------------------------------

def _proj_layout(W, nt):
    """W: [rows=nt*128, cols=nk*128] -> [nt, 128, cols]; slab t column-block ko
    holds W[t-block, ko-block].T so each [128,128] block is a matmul lhsT."""
    rows, cols = W.shape
    nk = cols // KP
    W4 = W.reshape(nt, KP, nk, KP)           # [t, i, ko, p]
    return np.ascontiguousarray(W4.transpose(0, 3, 2, 1).reshape(nt, KP, cols))


def _x_layout(x_sel):
    """x_sel: [C, H] -> [NH, 128, C] (x_sel.T tiled by h)."""
    C = x_sel.shape[0]
    return np.ascontiguousarray(x_sel.T.reshape(NH, KP, C))


# ---------------------------------------------------------------------------
# Main entry
# ---------------------------------------------------------------------------

def run_moe(x, W_router, W_gate, W_up, W_down, profile=False):
    import jax
    import jax.numpy as jnp
    from concourse.bass_utils import run_bass_kernel_spmd

    x = np.asarray(x, dtype=np.float32)
    W_router = np.asarray(W_router, dtype=np.float32)
    W_gate = np.asarray(W_gate, dtype=np.float32)
    W_up = np.asarray(W_up, dtype=np.float32)
    W_down = np.asarray(W_down, dtype=np.float32)

    b, s, h = x.shape
    n = b * s
    x_flat = x.reshape(n, h)

    # --- Router: replicate the reference's ops eagerly on the default backend
    # so the discrete top-k selection matches it bit-for-bit.
    xj = jnp.asarray(x_flat)
    probs = jax.nn.softmax(xj @ jnp.asarray(W_router).T, axis=-1)
    topk_w, topk_idx = jax.lax.top_k(probs, TOPK)
    topk_w = topk_w / topk_w.sum(axis=-1, keepdims=True)
    onehot = jax.nn.one_hot(topk_idx, E, dtype=xj.dtype)
    counts = onehot.sum(axis=1)
    aux_loss = jnp.var(counts.mean(axis=0), ddof=1) * E

    topk_idx_np = np.asarray(topk_idx)       # [N, 2] int32
    topk_w_np = np.asarray(topk_w)           # [N, 2] f32 (renormalized)

    # --- Dispatch (host-side sharding): per-expert token lists + capacity pad
    idx_lists, w_lists = [], []
    for e in range(E):
        sel = np.nonzero((topk_idx_np == e).any(axis=1))[0]
        we = np.where(topk_idx_np[sel, 0] == e,
                      topk_w_np[sel, 0], topk_w_np[sel, 1]).astype(np.float32)
        idx_lists.append(sel)
        w_lists.append(we)
    max_cnt = max(len(ix) for ix in idx_lists)
    C = max(KP, ((max_cnt + 7) // 8) * 8)

    nc = _get_compiled(C)

    in_maps = []
    for e in range(E):
        sel, we = idx_lists[e], w_lists[e]
        cnt = len(sel)
        x_sel = np.zeros((C, H), dtype=np.float32)
        x_sel[:cnt] = x_flat[sel]
        scrow = np.zeros((1, C), dtype=np.float32)
        scrow[0, :cnt] = we
        in_maps.append({
            "xg": _x_layout(x_sel),
            "sc": scrow,
            "wg": _proj_layout(W_gate[e], NI),
            "wu": _proj_layout(W_up[e], NI),
            "wd": _proj_layout(W_down[e], NH),
        })

    prof_times = None
    if profile:
        import profiling
        results, prof_times, neff_dir = profiling.run_profiled(
            nc, in_maps, list(range(N_CORES)))
    else:
        results = run_bass_kernel_spmd(nc, in_maps, list(range(N_CORES))).results

    # --- Combine (host-side unshard): scatter-add in expert order
    out_flat = np.zeros((n, H), dtype=np.float32)
    for e in range(E):
        sel = idx_lists[e]
        cnt = len(sel)
        d_e = results[e]["outT"].reshape(H, C).T   # [C, H]
        out_flat[sel] += d_e[:cnt]

    out = out_flat.reshape(b, s, h)
    return (out, np.asarray(aux_loss)), prof_times


def kernel(x, W_router, W_gate, W_up, W_down):
    (out, aux_loss), _ = run_moe(x, W_router, W_gate, W_up, W_down)
    return out, aux_loss


# revision 5
# speedup vs baseline: 1.1959x; 1.0290x over previous
"""MoE FFN (SwiGLU, top-2 of 8 experts) on 8 Trainium2 NeuronCores.

Strategy (expert-parallel, per sharding hint):
  - Router is replicated bit-exactly with the reference via eager jnp ops on
    the default jax backend; the top-2 selection and combine weights therefore
    match the reference's routing decisions exactly.
  - Token dispatch (the "all-to-all") happens host-side as the sharding step:
    each of the 8 cores receives exactly the tokens routed to its expert
    (padded to a common capacity C), with the expert's weights pre-transposed
    into PE-friendly tiled layouts.
  - Each core runs a dense SwiGLU FFN over its C tokens in float32r (fp32
    storage, full-rate PE) with fp32 PSUM accumulation.
  - The up-projection input is pre-scaled by the per-token combine weight, so
    the core's output is already the weighted expert contribution; the host
    scatter-adds the 8 outputs back into token order (expert order preserved).

Computation per core e (feature-on-partition layout, C gathered tokens):
    g.T = WgT.T @ xTg          [I, C]
    u.T = WuT.T @ (w * x).T    [I, C]
    a.T = silu(g.T) * u.T      [I, C]
    d.T = WdT.T @ a.T          [H, C]   -> outT (scatter-added on host)
"""

from contextlib import ExitStack

import numpy as np

B, S, H, I, E, TOPK = 2, 1024, 1024, 2048, 8, 2
N = B * S
KP = 128
NH = H // KP   # 8 h-tiles
NI = I // KP   # 16 i-tiles
N_CORES = 8

_COMPILED = {}  # C -> bass module


# ---------------------------------------------------------------------------
# Bass/Tile kernel
# ---------------------------------------------------------------------------

def _chunk_sizes(C, max_chunk=512):
    out, n0 = [], 0
    while n0 < C:
        sz = min(max_chunk, C - n0)
        out.append((n0, sz))
        n0 += sz
    return out


# This walrus build accepts only one semaphore wait on most engine
# instructions; Tile can emit more. Move the excess onto preceding sequencer
# NoOps on the same engine (equivalent blocking, engines execute in order).
_WAIT_LIMITS = {
    "InstTensorTensor": 1,
    "InstTensorCopy": 1,
    "InstActivation": 1,
    "InstMatmult": 1,
    "InstLdweights": 1,
    "InstMemset": 1,
    "InstTensorReduce": 1,
    "InstTensorScalarPtr": 1,
    "InstIota": 1,
    "InstDMACopy": 1,
    "InstDrain": 1,
    "InstEventSemaphore": 1,
}


def _legalize_waits(nc, mybir):
    nop_id = 0
    for f in nc.m.functions:
        for b in f.blocks:
            insts = list(b.instructions)
            out, changed = [], False
            for i in insts:
                lim = _WAIT_LIMITS.get(type(i).__name__)
                si = i.sync_info
                if lim is not None and si is not None and si.on_wait and len(si.on_wait) > lim:
                    waits = list(si.on_wait)
                    for w in waits[:-lim]:
                        nop_id += 1
                        nop = mybir.InstNoOp(name=f"I-waitnop-{nop_id}")
                        nop.engine = i.engine
                        nop.sync_info = mybir.SyncInfo(on_wait=[w], on_update=[])
                        out.append(nop)
                    si.on_wait = waits[-lim:]
                    changed = True
                out.append(i)
            if changed:
                b.instructions = out


def _build_moe_ffn(C, even_chunks=True, warmup_mms=40):
    import concourse.bass as bass
    import concourse.mybir as mybir
    import concourse.tile as tile

    F32 = mybir.dt.float32
    F32R = mybir.dt.float32r
    AF = mybir.ActivationFunctionType

    nc = bass.Bass()
    xg = nc.declare_dram_parameter("xg", [NH, KP, C], F32R, isOutput=False)
    sc = nc.declare_dram_parameter("sc", [1, C], F32, isOutput=False)
    wg = nc.declare_dram_parameter("wg", [NI, KP, H], F32R, isOutput=False)
    wu = nc.declare_dram_parameter("wu", [NI, KP, H], F32R, isOutput=False)
    wd = nc.declare_dram_parameter("wd", [NH, KP, I], F32R, isOutput=False)
    outT = nc.declare_dram_parameter("outT", [NH, KP, C], F32, isOutput=True)

    if even_chunks and C > 512:
        nck = (C + 511) // 512
        base = (C // nck) // KP * KP
        szs, left = [], C
        for i in range(nck - 1):
            szs.append(base)
            left -= base
        szs.append(left)
        chunks, n0 = [], 0
        for sz in szs:
            chunks.append((n0, sz))
            n0 += sz
    else:
        chunks = _chunk_sizes(C)

    with tile.TileContext(nc) as tc, ExitStack() as ctx:
        xpool = ctx.enter_context(tc.tile_pool(name="x", bufs=1))
        apool = ctx.enter_context(tc.tile_pool(name="act", bufs=1))
        spool = ctx.enter_context(tc.tile_pool(name="scale", bufs=1))
        wgp = ctx.enter_context(tc.tile_pool(name="wg", bufs=3))
        wup = ctx.enter_context(tc.tile_pool(name="wu", bufs=3))
        wdp = ctx.enter_context(tc.tile_pool(name="wd", bufs=NH))
        tmp = ctx.enter_context(tc.tile_pool(name="tmp", bufs=3))
        opool = ctx.enter_context(tc.tile_pool(name="o", bufs=3))
        pgp = ctx.enter_context(tc.tile_pool(name="pg", bufs=2, space="PSUM"))
        pup = ctx.enter_context(tc.tile_pool(name="pu", bufs=2, space="PSUM"))
        pdp = ctx.enter_context(tc.tile_pool(name="pd", bufs=3, space="PSUM"))
        pwp = ctx.enter_context(tc.tile_pool(name="pw", bufs=1, space="PSUM"))

        # PE warm-up during the initial DMA window: fp32 matmuls on memset
        # tiles keep the HAM activity window busy so real matmuls start at
        # full clock.
        if warmup_mms:
            wa = tmp.tile([KP, KP], F32, tag="wa")
            wb = tmp.tile([KP, 64], F32, tag="wb")
            nc.vector.memset(wa, 0.0)
            nc.vector.memset(wb, 0.0)
            pw = pwp.tile([KP, 64], F32, tag="pw")
            for i in range(warmup_mms):
                nc.tensor.matmul(pw, lhsT=wa, rhs=wb,
                                 start=(i == 0), stop=(i == warmup_mms - 1))

        # Weight slab for t=0 queued before the x tiles so the first matmuls
        # can start as soon as xg[0] lands.
        wg_sb0 = wgp.tile([KP, H], F32R, tag="wg")
        nc.sync.dma_start(out=wg_sb0, in_=wg[0])
        wu_sb0 = wup.tile([KP, H], F32R, tag="wu")
        nc.sync.dma_start(out=wu_sb0, in_=wu[0])

        xg_sb = xpool.tile([KP, NH * C], F32R, tag="xg")
        for ko in range(NH):
            nc.sync.dma_start(out=xg_sb[:, ko * C:(ko + 1) * C], in_=xg[ko])

        scale_sb = spool.tile([KP, C], F32, tag="sc")
        sc_b = bass.AP(tensor=sc[:].tensor, offset=0, ap=[[0, KP], [1, C]])
        nc.gpsimd.dma_start(out=scale_sb, in_=sc_b)

        act_sb = apool.tile([KP, NI * C], F32R, tag="a")

        # Phase A: gate/up projections + SwiGLU, one i-tile at a time
        for t in range(NI):
            if t == 0:
                wg_sb, wu_sb = wg_sb0, wu_sb0
            else:
                wg_sb = wgp.tile([KP, H], F32R, tag="wg")
                nc.sync.dma_start(out=wg_sb, in_=wg[t])
                wu_sb = wup.tile([KP, H], F32R, tag="wu")
                nc.sync.dma_start(out=wu_sb, in_=wu[t])
            for (n0, nsz) in chunks:
                pg = pgp.tile([KP, nsz], F32, tag="pg")
                pu = pup.tile([KP, nsz], F32, tag="pu")
                for ko in range(NH):
                    nc.tensor.matmul(
                        pg,
                        lhsT=wg_sb[:, ko * KP:(ko + 1) * KP],
                        rhs=xg_sb[:, ko * C + n0:ko * C + n0 + nsz],
                        start=(ko == 0), stop=(ko == NH - 1),
                    )
                for ko in range(NH):
                    nc.tensor.matmul(
                        pu,
                        lhsT=wu_sb[:, ko * KP:(ko + 1) * KP],
                        rhs=xg_sb[:, ko * C + n0:ko * C + n0 + nsz],
                        start=(ko == 0), stop=(ko == NH - 1),
                    )
                # Both mul operands produced by ACT so the DVE mul needs only
                # one semaphore wait (walrus limit).
                silu_t = tmp.tile([KP, nsz], F32, tag="silu")
                nc.scalar.activation(silu_t, pg, AF.Silu)
                u_t = tmp.tile([KP, nsz], F32, tag="ut")
                nc.scalar.activation(u_t, pu, AF.Copy)
                nc.vector.tensor_mul(act_sb[:, t * C + n0:t * C + n0 + nsz], silu_t, u_t)

        # Phase B: down projection, output scaled by per-token combine weight.
        # All slabs prefetched (they stream in behind phase A's weight loads).
        wd_sbs = []
        for ho in range(NH):
            wd_sb = wdp.tile([KP, I], F32R, tag="wd")
            nc.sync.dma_start(out=wd_sb, in_=wd[ho])
            wd_sbs.append(wd_sb)
        for ho in range(NH):
            wd_sb = wd_sbs[ho]
            for (n0, nsz) in chunks:
                pd = pdp.tile([KP, nsz], F32, tag="pd")
                for ko in range(NI):
                    nc.tensor.matmul(
                        pd,
                        lhsT=wd_sb[:, ko * KP:(ko + 1) * KP],
                        rhs=act_sb[:, ko * C + n0:ko * C + n0 + nsz],
                        start=(ko == 0), stop=(ko == NI - 1),
                    )
                o_sb = opool.tile([KP, nsz], F32, tag="o")
                nc.vector.tensor_mul(o_sb, pd, scale_sb[:, n0:n0 + nsz])
                nc.sync.dma_start(out=outT[ho, :, n0:n0 + nsz], in_=o_sb)

    _legalize_waits(nc, mybir)
    return nc


def _get_compiled(C):
    if C not in _COMPILED:
        _COMPILED[C] = _build_moe_ffn(C)
    return _COMPILED[C]


# ---------------------------------------------------------------------------
# Host-side layouts
# ---------------------------------------------------------------------------

def _proj_layout(W, nt):
    """W: [rows=nt*128, cols=nk*128] -> [nt, 128, cols]; slab t column-block ko
    holds W[t-block, ko-block].T so each [128,128] block is a matmul lhsT."""
    rows, cols = W.shape
    nk = cols // KP
    W4 = W.reshape(nt, KP, nk, KP)           # [t, i, ko, p]
    return np.ascontiguousarray(W4.transpose(0, 3, 2, 1).reshape(nt, KP, cols))


def _x_layout(x_sel):
    """x_sel: [C, H] -> [NH, 128, C] (x_sel.T tiled by h)."""
    C = x_sel.shape[0]
    return np.ascontiguousarray(x_sel.T.reshape(NH, KP, C))


# ---------------------------------------------------------------------------
# Main entry
# ---------------------------------------------------------------------------

def run_moe(x, W_router, W_gate, W_up, W_down, profile=False):
    import jax
    import jax.numpy as jnp
    from concourse.bass_utils import run_bass_kernel_spmd

    x = np.asarray(x, dtype=np.float32)
    W_router = np.asarray(W_router, dtype=np.float32)
    W_gate = np.asarray(W_gate, dtype=np.float32)
    W_up = np.asarray(W_up, dtype=np.float32)
    W_down = np.asarray(W_down, dtype=np.float32)

    b, s, h = x.shape
    n = b * s
    x_flat = x.reshape(n, h)

    # --- Router: replicate the reference's ops eagerly on the default backend
    # so the discrete top-k selection matches it bit-for-bit.
    xj = jnp.asarray(x_flat)
    probs = jax.nn.softmax(xj @ jnp.asarray(W_router).T, axis=-1)
    topk_w, topk_idx = jax.lax.top_k(probs, TOPK)
    topk_w = topk_w / topk_w.sum(axis=-1, keepdims=True)
    onehot = jax.nn.one_hot(topk_idx, E, dtype=xj.dtype)
    counts = onehot.sum(axis=1)
    aux_loss = jnp.var(counts.mean(axis=0), ddof=1) * E

    topk_idx_np = np.asarray(topk_idx)       # [N, 2] int32
    topk_w_np = np.asarray(topk_w)           # [N, 2] f32 (renormalized)

    # --- Dispatch (host-side sharding): per-expert token lists + capacity pad
    idx_lists, w_lists = [], []
    for e in range(E):
        sel = np.nonzero((topk_idx_np == e).any(axis=1))[0]
        we = np.where(topk_idx_np[sel, 0] == e,
                      topk_w_np[sel, 0], topk_w_np[sel, 1]).astype(np.float32)
        idx_lists.append(sel)
        w_lists.append(we)
    max_cnt = max(len(ix) for ix in idx_lists)
    C = max(KP, ((max_cnt + 7) // 8) * 8)

    nc = _get_compiled(C)

    in_maps = []
    for e in range(E):
        sel, we = idx_lists[e], w_lists[e]
        cnt = len(sel)
        x_sel = np.zeros((C, H), dtype=np.float32)
        x_sel[:cnt] = x_flat[sel]
        scrow = np.zeros((1, C), dtype=np.float32)
        scrow[0, :cnt] = we
        in_maps.append({
            "xg": _x_layout(x_sel),
            "sc": scrow,
            "wg": _proj_layout(W_gate[e], NI),
            "wu": _proj_layout(W_up[e], NI),
            "wd": _proj_layout(W_down[e], NH),
        })

    prof_times = None
    if profile:
        import profiling
        results, prof_times, neff_dir = profiling.run_profiled(
            nc, in_maps, list(range(N_CORES)))
    else:
        results = run_bass_kernel_spmd(nc, in_maps, list(range(N_CORES))).results

    # --- Combine (host-side unshard): scatter-add in expert order
    out_flat = np.zeros((n, H), dtype=np.float32)
    for e in range(E):
        sel = idx_lists[e]
        cnt = len(sel)
        d_e = results[e]["outT"].reshape(H, C).T   # [C, H]
        out_flat[sel] += d_e[:cnt]

    out = out_flat.reshape(b, s, h)
    return (out, np.asarray(aux_loss)), prof_times


def kernel(x, W_router, W_gate, W_up, W_down):
    (out, aux_loss), _ = run_moe(x, W_router, W_gate, W_up, W_down)
    return out, aux_loss


# revision 7
# speedup vs baseline: 1.3092x; 1.0947x over previous
"""MoE FFN (SwiGLU, top-2 of 8 experts) on 8 Trainium2 NeuronCores.

Strategy (expert-parallel, per sharding hint):
  - Router is replicated bit-exactly with the reference via eager jnp ops on
    the default jax backend; the top-2 selection and combine weights therefore
    match the reference's routing decisions exactly.
  - Token dispatch (the "all-to-all") happens host-side as the sharding step:
    each of the 8 cores receives exactly the tokens routed to its expert
    (padded to a common capacity C), with the expert's weights pre-transposed
    into PE-friendly tiled layouts.
  - Each core runs a dense SwiGLU FFN over its C tokens in float32r (fp32
    storage, full-rate PE) with fp32 PSUM accumulation.
  - The up-projection input is pre-scaled by the per-token combine weight, so
    the core's output is already the weighted expert contribution; the host
    scatter-adds the 8 outputs back into token order (expert order preserved).

Computation per core e (feature-on-partition layout, C gathered tokens):
    g.T = WgT.T @ xTg          [I, C]
    u.T = WuT.T @ (w * x).T    [I, C]
    a.T = silu(g.T) * u.T      [I, C]
    d.T = WdT.T @ a.T          [H, C]   -> outT (scatter-added on host)
"""

from contextlib import ExitStack

import numpy as np

B, S, H, I, E, TOPK = 2, 1024, 1024, 2048, 8, 2
N = B * S
KP = 128
NH = H // KP   # 8 h-tiles
NI = I // KP   # 16 i-tiles
N_CORES = 8

_COMPILED = {}  # C -> bass module


# ---------------------------------------------------------------------------
# Bass/Tile kernel
# ---------------------------------------------------------------------------

def _chunk_sizes(C, max_chunk=512):
    out, n0 = [], 0
    while n0 < C:
        sz = min(max_chunk, C - n0)
        out.append((n0, sz))
        n0 += sz
    return out


# This walrus build accepts only one semaphore wait on most engine
# instructions; Tile can emit more. Move the excess onto preceding sequencer
# NoOps on the same engine (equivalent blocking, engines execute in order).
_WAIT_LIMITS = {
    "InstTensorTensor": 1,
    "InstTensorCopy": 1,
    "InstActivation": 1,
    "InstMatmult": 1,
    "InstLdweights": 1,
    "InstMemset": 1,
    "InstTensorReduce": 1,
    "InstTensorScalarPtr": 1,
    "InstIota": 1,
    "InstDMACopy": 1,
    "InstDrain": 1,
    "InstEventSemaphore": 1,
}


def _legalize_waits(nc, mybir):
    nop_id = 0
    for f in nc.m.functions:
        for b in f.blocks:
            insts = list(b.instructions)
            out, changed = [], False
            for i in insts:
                lim = _WAIT_LIMITS.get(type(i).__name__)
                si = i.sync_info
                if lim is not None and si is not None and si.on_wait and len(si.on_wait) > lim:
                    waits = list(si.on_wait)
                    for w in waits[:-lim]:
                        nop_id += 1
                        nop = mybir.InstNoOp(name=f"I-waitnop-{nop_id}")
                        nop.engine = i.engine
                        nop.sync_info = mybir.SyncInfo(on_wait=[w], on_update=[])
                        out.append(nop)
                    si.on_wait = waits[-lim:]
                    changed = True
                out.append(i)
            if changed:
                b.instructions = out


def _build_moe_ffn(C, even_chunks=True, warmup_mms=16):
    import concourse.bass as bass
    import concourse.mybir as mybir
    import concourse.tile as tile

    F32 = mybir.dt.float32
    F32R = mybir.dt.float32r
    AF = mybir.ActivationFunctionType

    nc = bass.Bass()
    xg = nc.declare_dram_parameter("xg", [NH, KP, C], F32R, isOutput=False)
    sc = nc.declare_dram_parameter("sc", [1, C], F32, isOutput=False)
    wg = nc.declare_dram_parameter("wg", [NI, KP, H], F32R, isOutput=False)
    wu = nc.declare_dram_parameter("wu", [NI, KP, H], F32R, isOutput=False)
    wd = nc.declare_dram_parameter("wd", [NH, KP, I], F32R, isOutput=False)
    outT = nc.declare_dram_parameter("outT", [NH, KP, C], F32, isOutput=True)

    if even_chunks and C > 512:
        nck = (C + 511) // 512
        base = (C // nck) // KP * KP
        szs, left = [], C
        for i in range(nck - 1):
            szs.append(base)
            left -= base
        szs.append(left)
        chunks, n0 = [], 0
        for sz in szs:
            chunks.append((n0, sz))
            n0 += sz
    else:
        chunks = _chunk_sizes(C)

    with tile.TileContext(nc) as tc, ExitStack() as ctx:
        xpool = ctx.enter_context(tc.tile_pool(name="x", bufs=1))
        apool = ctx.enter_context(tc.tile_pool(name="act", bufs=1))
        spool = ctx.enter_context(tc.tile_pool(name="scale", bufs=1))
        wgp = ctx.enter_context(tc.tile_pool(name="wg", bufs=3))
        wup = ctx.enter_context(tc.tile_pool(name="wu", bufs=3))
        wdp = ctx.enter_context(tc.tile_pool(name="wd", bufs=NH))
        tmp = ctx.enter_context(tc.tile_pool(name="tmp", bufs=3))
        opool = ctx.enter_context(tc.tile_pool(name="o", bufs=3))
        pgp = ctx.enter_context(tc.tile_pool(name="pg", bufs=2, space="PSUM"))
        pup = ctx.enter_context(tc.tile_pool(name="pu", bufs=2, space="PSUM"))
        pdp = ctx.enter_context(tc.tile_pool(name="pd", bufs=3, space="PSUM"))
        pwp = ctx.enter_context(tc.tile_pool(name="pw", bufs=1, space="PSUM"))

        # PE warm-up during the initial DMA window: fp32 matmuls on memset
        # tiles keep the HAM activity window busy so real matmuls start at
        # full clock.
        if warmup_mms:
            wa = tmp.tile([KP, KP], F32, tag="wa")
            wb = tmp.tile([KP, 64], F32, tag="wb")
            nc.vector.memset(wa, 0.0)
            nc.vector.memset(wb, 0.0)
            pw = pwp.tile([KP, 64], F32, tag="pw")
            for i in range(warmup_mms):
                nc.tensor.matmul(pw, lhsT=wa, rhs=wb,
                                 start=(i == 0), stop=(i == warmup_mms - 1))

        # Weight slab for t=0 queued before the x tiles so the first matmuls
        # can start as soon as xg[0] lands.
        wg_sb0 = wgp.tile([KP, H], F32R, tag="wg")
        nc.sync.dma_start(out=wg_sb0, in_=wg[0])
        wu_sb0 = wup.tile([KP, H], F32R, tag="wu")
        nc.sync.dma_start(out=wu_sb0, in_=wu[0])

        xg_sbs = []
        for ko in range(NH):
            xg_t = xpool.tile([KP, C], F32R, tag=f"xg{ko}")
            nc.sync.dma_start(out=xg_t, in_=xg[ko])
            xg_sbs.append(xg_t)

        scale_sb = spool.tile([KP, C], F32, tag="sc")
        sc_b = bass.AP(tensor=sc[:].tensor, offset=0, ap=[[0, KP], [1, C]])
        nc.gpsimd.dma_start(out=scale_sb, in_=sc_b)

        act_sbs = []
        for t in range(NI):
            act_t = apool.tile([KP, C], F32R, tag=f"a{t}")
            act_sbs.append(act_t)

        # Phase A: gate/up projections + SwiGLU, one i-tile at a time
        for t in range(NI):
            if t == 0:
                wg_sb, wu_sb = wg_sb0, wu_sb0
            else:
                wg_sb = wgp.tile([KP, H], F32R, tag="wg")
                nc.sync.dma_start(out=wg_sb, in_=wg[t])
                wu_sb = wup.tile([KP, H], F32R, tag="wu")
                nc.sync.dma_start(out=wu_sb, in_=wu[t])
            for (n0, nsz) in chunks:
                pg = pgp.tile([KP, nsz], F32, tag="pg")
                pu = pup.tile([KP, nsz], F32, tag="pu")
                for ko in range(NH):
                    nc.tensor.matmul(
                        pg,
                        lhsT=wg_sb[:, ko * KP:(ko + 1) * KP],
                        rhs=xg_sbs[ko][:, n0:n0 + nsz],
                        start=(ko == 0), stop=(ko == NH - 1),
                    )
                for ko in range(NH):
                    nc.tensor.matmul(
                        pu,
                        lhsT=wu_sb[:, ko * KP:(ko + 1) * KP],
                        rhs=xg_sbs[ko][:, n0:n0 + nsz],
                        start=(ko == 0), stop=(ko == NH - 1),
                    )
                # Both mul operands produced by ACT so the DVE mul needs only
                # one semaphore wait (walrus limit).
                silu_t = tmp.tile([KP, nsz], F32, tag="silu")
                nc.scalar.activation(silu_t, pg, AF.Silu)
                u_t = tmp.tile([KP, nsz], F32, tag="ut")
                nc.scalar.activation(u_t, pu, AF.Copy)
                nc.vector.tensor_mul(act_sbs[t][:, n0:n0 + nsz], silu_t, u_t)

        # Phase B: down projection, output scaled by per-token combine weight.
        # All slabs prefetched (they stream in behind phase A's weight loads).
        wd_sbs = []
        for ho in range(NH):
            wd_sb = wdp.tile([KP, I], F32R, tag="wd")
            nc.sync.dma_start(out=wd_sb, in_=wd[ho])
            wd_sbs.append(wd_sb)
        for ho in range(NH):
            wd_sb = wd_sbs[ho]
            for (n0, nsz) in chunks:
                pd = pdp.tile([KP, nsz], F32, tag="pd")
                for ko in range(NI):
                    nc.tensor.matmul(
                        pd,
                        lhsT=wd_sb[:, ko * KP:(ko + 1) * KP],
                        rhs=act_sbs[ko][:, n0:n0 + nsz],
                        start=(ko == 0), stop=(ko == NI - 1),
                    )
                o_sb = opool.tile([KP, nsz], F32, tag="o")
                nc.vector.tensor_mul(o_sb, pd, scale_sb[:, n0:n0 + nsz])
                nc.sync.dma_start(out=outT[ho, :, n0:n0 + nsz], in_=o_sb)

    _legalize_waits(nc, mybir)
    return nc


def _get_compiled(C):
    if C not in _COMPILED:
        _COMPILED[C] = _build_moe_ffn(C)
    return _COMPILED[C]


# ---------------------------------------------------------------------------
# Host-side layouts
# ---------------------------------------------------------------------------

def _proj_layout(W, nt):
    """W: [rows=nt*128, cols=nk*128] -> [nt, 128, cols]; slab t column-block ko
    holds W[t-block, ko-block].T so each [128,128] block is a matmul lhsT."""
    rows, cols = W.shape
    nk = cols // KP
    W4 = W.reshape(nt, KP, nk, KP)           # [t, i, ko, p]
    return np.ascontiguousarray(W4.transpose(0, 3, 2, 1).reshape(nt, KP, cols))


def _x_layout(x_sel):
    """x_sel: [C, H] -> [NH, 128, C] (x_sel.T tiled by h)."""
    C = x_sel.shape[0]
    return np.ascontiguousarray(x_sel.T.reshape(NH, KP, C))


# ---------------------------------------------------------------------------
# Main entry
# ---------------------------------------------------------------------------

def run_moe(x, W_router, W_gate, W_up, W_down, profile=False):
    import jax
    import jax.numpy as jnp
    from concourse.bass_utils import run_bass_kernel_spmd

    x = np.asarray(x, dtype=np.float32)
    W_router = np.asarray(W_router, dtype=np.float32)
    W_gate = np.asarray(W_gate, dtype=np.float32)
    W_up = np.asarray(W_up, dtype=np.float32)
    W_down = np.asarray(W_down, dtype=np.float32)

    b, s, h = x.shape
    n = b * s
    x_flat = x.reshape(n, h)

    # --- Router: replicate the reference's ops eagerly on the default backend
    # so the discrete top-k selection matches it bit-for-bit.
    xj = jnp.asarray(x_flat)
    probs = jax.nn.softmax(xj @ jnp.asarray(W_router).T, axis=-1)
    topk_w, topk_idx = jax.lax.top_k(probs, TOPK)
    topk_w = topk_w / topk_w.sum(axis=-1, keepdims=True)
    onehot = jax.nn.one_hot(topk_idx, E, dtype=xj.dtype)
    counts = onehot.sum(axis=1)
    aux_loss = jnp.var(counts.mean(axis=0), ddof=1) * E

    topk_idx_np = np.asarray(topk_idx)       # [N, 2] int32
    topk_w_np = np.asarray(topk_w)           # [N, 2] f32 (renormalized)

    # --- Dispatch (host-side sharding): per-expert token lists + capacity pad
    idx_lists, w_lists = [], []
    for e in range(E):
        sel = np.nonzero((topk_idx_np == e).any(axis=1))[0]
        we = np.where(topk_idx_np[sel, 0] == e,
                      topk_w_np[sel, 0], topk_w_np[sel, 1]).astype(np.float32)
        idx_lists.append(sel)
        w_lists.append(we)
    max_cnt = max(len(ix) for ix in idx_lists)
    C = max(KP, ((max_cnt + 7) // 8) * 8)

    nc = _get_compiled(C)

    in_maps = []
    for e in range(E):
        sel, we = idx_lists[e], w_lists[e]
        cnt = len(sel)
        x_sel = np.zeros((C, H), dtype=np.float32)
        x_sel[:cnt] = x_flat[sel]
        scrow = np.zeros((1, C), dtype=np.float32)
        scrow[0, :cnt] = we
        in_maps.append({
            "xg": _x_layout(x_sel),
            "sc": scrow,
            "wg": _proj_layout(W_gate[e], NI),
            "wu": _proj_layout(W_up[e], NI),
            "wd": _proj_layout(W_down[e], NH),
        })

    prof_times = None
    if profile:
        import profiling
        results, prof_times, neff_dir = profiling.run_profiled(
            nc, in_maps, list(range(N_CORES)))
    else:
        results = run_bass_kernel_spmd(nc, in_maps, list(range(N_CORES))).results

    # --- Combine (host-side unshard): scatter-add in expert order
    out_flat = np.zeros((n, H), dtype=np.float32)
    for e in range(E):
        sel = idx_lists[e]
        cnt = len(sel)
        d_e = results[e]["outT"].reshape(H, C).T   # [C, H]
        out_flat[sel] += d_e[:cnt]

    out = out_flat.reshape(b, s, h)
    return (out, np.asarray(aux_loss)), prof_times


def kernel(x, W_router, W_gate, W_up, W_down):
    (out, aux_loss), _ = run_moe(x, W_router, W_gate, W_up, W_down)
    return out, aux_loss


# revision 8
# speedup vs baseline: 1.3773x; 1.0521x over previous
"""MoE FFN (SwiGLU, top-2 of 8 experts) on 8 Trainium2 NeuronCores.

Strategy (expert-parallel, per sharding hint):
  - Router is replicated bit-exactly with the reference via eager jnp ops on
    the default jax backend; the top-2 selection and combine weights therefore
    match the reference's routing decisions exactly.
  - Token dispatch (the "all-to-all") happens host-side as the sharding step:
    each of the 8 cores receives exactly the tokens routed to its expert
    (padded to a common capacity C), with the expert's weights pre-transposed
    into PE-friendly tiled layouts.
  - Each core runs a dense SwiGLU FFN over its C tokens in float32r (fp32
    storage, full-rate PE) with fp32 PSUM accumulation.
  - The up-projection input is pre-scaled by the per-token combine weight, so
    the core's output is already the weighted expert contribution; the host
    scatter-adds the 8 outputs back into token order (expert order preserved).

Computation per core e (feature-on-partition layout, C gathered tokens):
    g.T = WgT.T @ xTg          [I, C]
    u.T = WuT.T @ (w * x).T    [I, C]
    a.T = silu(g.T) * u.T      [I, C]
    d.T = WdT.T @ a.T          [H, C]   -> outT (scatter-added on host)
"""

from contextlib import ExitStack

import numpy as np

B, S, H, I, E, TOPK = 2, 1024, 1024, 2048, 8, 2
N = B * S
KP = 128
NH = H // KP   # 8 h-tiles
NI = I // KP   # 16 i-tiles
N_CORES = 8

_COMPILED = {}  # C -> bass module


# ---------------------------------------------------------------------------
# Bass/Tile kernel
# ---------------------------------------------------------------------------

def _chunk_sizes(C, max_chunk=512):
    out, n0 = [], 0
    while n0 < C:
        sz = min(max_chunk, C - n0)
        out.append((n0, sz))
        n0 += sz
    return out


# This walrus build accepts only one semaphore wait on most engine
# instructions; Tile can emit more. Move the excess onto preceding sequencer
# NoOps on the same engine (equivalent blocking, engines execute in order).
_WAIT_LIMITS = {
    "InstTensorTensor": 1,
    "InstTensorCopy": 1,
    "InstActivation": 1,
    "InstMatmult": 1,
    "InstLdweights": 1,
    "InstMemset": 1,
    "InstTensorReduce": 1,
    "InstTensorScalarPtr": 1,
    "InstIota": 1,
    "InstDMACopy": 1,
    "InstDrain": 1,
    "InstEventSemaphore": 1,
}


def _legalize_waits(nc, mybir):
    nop_id = 0
    for f in nc.m.functions:
        for b in f.blocks:
            insts = list(b.instructions)
            out, changed = [], False
            for i in insts:
                lim = _WAIT_LIMITS.get(type(i).__name__)
                si = i.sync_info
                if lim is not None and si is not None and si.on_wait and len(si.on_wait) > lim:
                    waits = list(si.on_wait)
                    for w in waits[:-lim]:
                        nop_id += 1
                        nop = mybir.InstNoOp(name=f"I-waitnop-{nop_id}")
                        nop.engine = i.engine
                        nop.sync_info = mybir.SyncInfo(on_wait=[w], on_update=[])
                        out.append(nop)
                    si.on_wait = waits[-lim:]
                    changed = True
                out.append(i)
            if changed:
                b.instructions = out


def _build_moe_ffn(C, even_chunks=True, warmup_mms=16):
    import concourse.bass as bass
    import concourse.mybir as mybir
    import concourse.tile as tile

    F32 = mybir.dt.float32
    F32R = mybir.dt.float32r
    AF = mybir.ActivationFunctionType

    nc = bass.Bass()
    xg = nc.declare_dram_parameter("xg", [NH, KP, C], F32R, isOutput=False)
    wg = nc.declare_dram_parameter("wg", [NI, KP, H], F32R, isOutput=False)
    wu = nc.declare_dram_parameter("wu", [NI, KP, H], F32R, isOutput=False)
    wd = nc.declare_dram_parameter("wd", [NH, KP, I], F32R, isOutput=False)
    outT = nc.declare_dram_parameter("outT", [NH, KP, C], F32, isOutput=True)

    if even_chunks and C > 512:
        nck = (C + 511) // 512
        base = (C // nck) // KP * KP
        szs, left = [], C
        for i in range(nck - 1):
            szs.append(base)
            left -= base
        szs.append(left)
        chunks, n0 = [], 0
        for sz in szs:
            chunks.append((n0, sz))
            n0 += sz
    else:
        chunks = _chunk_sizes(C)

    with tile.TileContext(nc) as tc, ExitStack() as ctx:
        xpool = ctx.enter_context(tc.tile_pool(name="x", bufs=1))
        apool = ctx.enter_context(tc.tile_pool(name="act", bufs=1))
        wgp = ctx.enter_context(tc.tile_pool(name="wg", bufs=3))
        wup = ctx.enter_context(tc.tile_pool(name="wu", bufs=3))
        wdp = ctx.enter_context(tc.tile_pool(name="wd", bufs=NH))
        tmp = ctx.enter_context(tc.tile_pool(name="tmp", bufs=3))
        opool = ctx.enter_context(tc.tile_pool(name="o", bufs=3))
        pgp = ctx.enter_context(tc.tile_pool(name="pg", bufs=2, space="PSUM"))
        pup = ctx.enter_context(tc.tile_pool(name="pu", bufs=2, space="PSUM"))
        pdp = ctx.enter_context(tc.tile_pool(name="pd", bufs=4, space="PSUM"))

        # PE warm-up during the initial DMA window: fp32 matmuls on memset
        # tiles keep the HAM activity window busy so real matmuls start at
        # full clock.
        if warmup_mms:
            wa = tmp.tile([KP, KP], F32, tag="wa")
            wb = tmp.tile([KP, 64], F32, tag="wb")
            nc.vector.memset(wa, 0.0)
            nc.vector.memset(wb, 0.0)
            pw = pgp.tile([KP, 64], F32, tag="pg")
            for i in range(warmup_mms):
                nc.tensor.matmul(pw, lhsT=wa, rhs=wb,
                                 start=(i == 0), stop=(i == warmup_mms - 1))

        # Weight slab for t=0 queued before the x tiles so the first matmuls
        # can start as soon as xg[0] lands.
        wg_sb0 = wgp.tile([KP, H], F32R, tag="wg")
        nc.sync.dma_start(out=wg_sb0, in_=wg[0])
        wu_sb0 = wup.tile([KP, H], F32R, tag="wu")
        nc.sync.dma_start(out=wu_sb0, in_=wu[0])

        xg_sbs = []
        for ko in range(NH):
            xg_t = xpool.tile([KP, C], F32R, tag=f"xg{ko}")
            nc.sync.dma_start(out=xg_t, in_=xg[ko])
            xg_sbs.append(xg_t)

        act_sbs = []
        for t in range(NI):
            act_t = apool.tile([KP, C], F32R, tag=f"a{t}")
            act_sbs.append(act_t)

        # Phase A: gate/up projections + SwiGLU, one i-tile at a time
        for t in range(NI):
            if t == 0:
                wg_sb, wu_sb = wg_sb0, wu_sb0
            else:
                wg_sb = wgp.tile([KP, H], F32R, tag="wg")
                nc.sync.dma_start(out=wg_sb, in_=wg[t])
                wu_sb = wup.tile([KP, H], F32R, tag="wu")
                nc.sync.dma_start(out=wu_sb, in_=wu[t])
            for (n0, nsz) in chunks:
                pg = pgp.tile([KP, nsz], F32, tag="pg")
                pu = pup.tile([KP, nsz], F32, tag="pu")
                for ko in range(NH):
                    nc.tensor.matmul(
                        pg,
                        lhsT=wg_sb[:, ko * KP:(ko + 1) * KP],
                        rhs=xg_sbs[ko][:, n0:n0 + nsz],
                        start=(ko == 0), stop=(ko == NH - 1),
                    )
                for ko in range(NH):
                    nc.tensor.matmul(
                        pu,
                        lhsT=wu_sb[:, ko * KP:(ko + 1) * KP],
                        rhs=xg_sbs[ko][:, n0:n0 + nsz],
                        start=(ko == 0), stop=(ko == NH - 1),
                    )
                # Both mul operands produced by ACT so the DVE mul needs only
                # one semaphore wait (walrus limit).
                silu_t = tmp.tile([KP, nsz], F32, tag="silu")
                nc.scalar.activation(silu_t, pg, AF.Silu)
                u_t = tmp.tile([KP, nsz], F32, tag="ut")
                nc.scalar.activation(u_t, pu, AF.Copy)
                nc.vector.tensor_mul(act_sbs[t][:, n0:n0 + nsz], silu_t, u_t)

        # Phase B: down projection, output scaled by per-token combine weight.
        # All slabs prefetched (they stream in behind phase A's weight loads).
        wd_sbs = []
        for ho in range(NH):
            wd_sb = wdp.tile([KP, I], F32R, tag="wd")
            nc.sync.dma_start(out=wd_sb, in_=wd[ho])
            wd_sbs.append(wd_sb)
        for ho in range(NH):
            wd_sb = wd_sbs[ho]
            for (n0, nsz) in chunks:
                pd = pdp.tile([KP, nsz], F32, tag="pd")
                for ko in range(NI):
                    nc.tensor.matmul(
                        pd,
                        lhsT=wd_sb[:, ko * KP:(ko + 1) * KP],
                        rhs=act_sbs[ko][:, n0:n0 + nsz],
                        start=(ko == 0), stop=(ko == NI - 1),
                    )
                o_sb = opool.tile([KP, nsz], F32, tag="o")
                nc.vector.tensor_copy(o_sb, pd)
                nc.sync.dma_start(out=outT[ho, :, n0:n0 + nsz], in_=o_sb)

    _legalize_waits(nc, mybir)
    return nc


def _get_compiled(C):
    if C not in _COMPILED:
        _COMPILED[C] = _build_moe_ffn(C)
    return _COMPILED[C]


# ---------------------------------------------------------------------------
# Host-side layouts
# ---------------------------------------------------------------------------

def _proj_layout(W, nt):
    """W: [rows=nt*128, cols=nk*128] -> [nt, 128, cols]; slab t column-block ko
    holds W[t-block, ko-block].T so each [128,128] block is a matmul lhsT."""
    rows, cols = W.shape
    nk = cols // KP
    W4 = W.reshape(nt, KP, nk, KP)           # [t, i, ko, p]
    return np.ascontiguousarray(W4.transpose(0, 3, 2, 1).reshape(nt, KP, cols))


def _x_layout(x_sel):
    """x_sel: [C, H] -> [NH, 128, C] (x_sel.T tiled by h)."""
    C = x_sel.shape[0]
    return np.ascontiguousarray(x_sel.T.reshape(NH, KP, C))


# ---------------------------------------------------------------------------
# Main entry
# ---------------------------------------------------------------------------

def run_moe(x, W_router, W_gate, W_up, W_down, profile=False):
    import jax
    import jax.numpy as jnp
    from concourse.bass_utils import run_bass_kernel_spmd

    x = np.asarray(x, dtype=np.float32)
    W_router = np.asarray(W_router, dtype=np.float32)
    W_gate = np.asarray(W_gate, dtype=np.float32)
    W_up = np.asarray(W_up, dtype=np.float32)
    W_down = np.asarray(W_down, dtype=np.float32)

    b, s, h = x.shape
    n = b * s
    x_flat = x.reshape(n, h)

    # --- Router: replicate the reference's ops eagerly on the default backend
    # so the discrete top-k selection matches it bit-for-bit.
    xj = jnp.asarray(x_flat)
    probs = jax.nn.softmax(xj @ jnp.asarray(W_router).T, axis=-1)
    topk_w, topk_idx = jax.lax.top_k(probs, TOPK)
    topk_w = topk_w / topk_w.sum(axis=-1, keepdims=True)
    onehot = jax.nn.one_hot(topk_idx, E, dtype=xj.dtype)
    counts = onehot.sum(axis=1)
    aux_loss = jnp.var(counts.mean(axis=0), ddof=1) * E

    topk_idx_np = np.asarray(topk_idx)       # [N, 2] int32
    topk_w_np = np.asarray(topk_w)           # [N, 2] f32 (renormalized)

    # --- Dispatch (host-side sharding): per-expert token lists + capacity pad
    idx_lists, w_lists = [], []
    for e in range(E):
        sel = np.nonzero((topk_idx_np == e).any(axis=1))[0]
        we = np.where(topk_idx_np[sel, 0] == e,
                      topk_w_np[sel, 0], topk_w_np[sel, 1]).astype(np.float32)
        idx_lists.append(sel)
        w_lists.append(we)
    max_cnt = max(len(ix) for ix in idx_lists)
    C = max(KP, ((max_cnt + 7) // 8) * 8)

    nc = _get_compiled(C)

    in_maps = []
    for e in range(E):
        sel, we = idx_lists[e], w_lists[e]
        cnt = len(sel)
        x_sel = np.zeros((C, H), dtype=np.float32)
        x_sel[:cnt] = x_flat[sel]
        in_maps.append({
            "xg": _x_layout(x_sel),
            "wg": _proj_layout(W_gate[e], NI),
            "wu": _proj_layout(W_up[e], NI),
            "wd": _proj_layout(W_down[e], NH),
        })

    prof_times = None
    if profile:
        import profiling
        results, prof_times, neff_dir = profiling.run_profiled(
            nc, in_maps, list(range(N_CORES)))
    else:
        results = run_bass_kernel_spmd(nc, in_maps, list(range(N_CORES))).results

    # --- Combine (host-side unshard): scatter-add in expert order
    out_flat = np.zeros((n, H), dtype=np.float32)
    for e in range(E):
        sel, we = idx_lists[e], w_lists[e]
        cnt = len(sel)
        d_e = results[e]["outT"].reshape(H, C).T   # [C, H]
        out_flat[sel] += d_e[:cnt] * we[:, None]

    out = out_flat.reshape(b, s, h)
    return (out, np.asarray(aux_loss)), prof_times


def kernel(x, W_router, W_gate, W_up, W_down):
    (out, aux_loss), _ = run_moe(x, W_router, W_gate, W_up, W_down)
    return out, aux_loss
